# revision 9
# baseline (speedup 1.0000x reference)
"""Bass/Tile TRN2 kernel for nn_MultiHeadSeqAttention_82789789597729.

Math: the reference's softmax / positional scores are dead code -- its output
is exactly  out = concat_h(q_h @ k_h^T @ v_h) @ Wo^T  with no nonlinearity.
By associativity  q (k^T v)  replaces the [M,M] score matrix with a [D,D]
one, collapsing ~69 GFLOP to ~26 GFLOP.

Sharding: tensor-parallel over heads (4 heads / core) x data-parallel over
batch (B=2) -> 8 cores. Each core computes a full-M partial output for its
head group; the host sums the 4 partials per batch (row-parallel unshard).
"""

import numpy as np
import ml_dtypes

import concourse.bass as bass
import concourse.mybir as mybir
import concourse.tile as tile
from concourse.bass_utils import run_bass_kernel_spmd
from concourse.vector_clock import ScopedClock
import bass_rust

B, M, H, K, D = 2, 2048, 1024, 16, 64
N_CORES = 8
HPC = 4           # heads per core
CC = HPC * D      # 256 local feature columns per core
P = 128

# matmul dtype mode: "f32" (safe), "f32r" (full-rate fp32 storage), "bf16"
MM_DT = "f16"


# --- workaround: this walrus rejects multi-wait Drain instructions, so split
# --- the TileContext exit drain into one single-wait drain per proc.
def _split_drain_and_barrier(self, tick_clock, wait_clock):
    n_procs = len(list(tick_clock.global_clock))
    for p, t in enumerate(tick_clock.global_clock):
        if t <= 0:
            continue
        single = bass_rust.VectorClock(
            [t if i == p else 0 for i in range(n_procs)]
        )
        d = self.nc.sync.drain()
        wait_clock.add_sem_waits(d.ins, ScopedClock({None: single}))
    self.nc.all_engine_barrier()
    popped = self.nc._tile_sem_poison_stack.pop()
    assert popped is self._sem_poison
    self.nc.clear_and_free_semaphores(list(self.sems.allocated().values()))
    self.nc.all_engine_barrier()


# --- workaround: the same walrus caps sync waits at 1 per instruction
# --- (2 for EventSemaphore). Tile's wait-assignment can attach more; hoist
# --- the extras onto single-wait nop carriers emitted just before.
_ORIG_COMMIT_AND_LOWER = tile.TileContext._commit_and_lower


def _wait_split_commit_and_lower(self, inst, original_block, old_bb_map,
                                 bb_to_exit_bb):
    si = inst.sync_info
    cap = 2 if isinstance(inst, mybir.InstEventSemaphore) else 1
    ow = list(si.on_wait) if si is not None and si.on_wait else []
    if len(ow) > cap and inst.is_executable():
        for w in ow[:-cap]:
            carrier = self.nc.engines[inst.engine].nop(nofuse=True)
            carrier.ins.sync_info = bass_rust.SyncInfo(
                on_wait=[w], on_update=[]
            )
        inst.sync_info = bass_rust.SyncInfo(
            on_wait=ow[-cap:], on_update=list(si.on_update or [])
        )
    return _ORIG_COMMIT_AND_LOWER(
        self, inst, original_block, old_bb_map, bb_to_exit_bb
    )


if not getattr(tile.TileContext, "_split_drain_patched", False):
    tile.TileContext._drain_and_barrier = _split_drain_and_barrier
    tile.TileContext._commit_and_lower = _wait_split_commit_and_lower
    tile.TileContext._split_drain_patched = True


def _mm(nc, out, lhsT, rhs, start, stop):
    nc.tensor.matmul(out, lhsT, rhs, start=start, stop=stop)


def _build_nc():
    if MM_DT == "bf16":
        io_dt = mybir.dt.bfloat16
    elif MM_DT == "f16":
        io_dt = mybir.dt.float16
    elif MM_DT == "f32r":
        io_dt = mybir.dt.float32r
    else:
        io_dt = mybir.dt.float32
    f32 = mybir.dt.float32

    nc = bass.Bass()
    hT = nc.dram_tensor("hT", [H, M], io_dt, kind="ExternalInput")
    hcT = nc.dram_tensor("hcT", [H, M], io_dt, kind="ExternalInput")
    wqT = nc.dram_tensor("wqT", [H, CC], io_dt, kind="ExternalInput")
    wkvT = nc.dram_tensor("wkvT", [H, 2 * CC], io_dt, kind="ExternalInput")
    woT = nc.dram_tensor("woT", [CC, H], io_dt, kind="ExternalInput")
    outp = nc.dram_tensor("out", [M, H], f32, kind="ExternalOutput")

    IT = H // P           # 8 contraction tiles over feature dim
    LT = M // P           # 16 tiles over sequence dim
    MC = M // 512         # 4 moving chunks over sequence dim
    DT = CC // P          # 2 partition tiles over local feature cols
    JC = H // 512         # 2 chunks over output feature dim

    x_bufs = 12 if MM_DT in ("bf16", "f16") else 9

    with tile.TileContext(nc) as tc:
        with (
            tc.tile_pool(name="wp", bufs=1) as wp,
            tc.tile_pool(name="xp", bufs=16) as xp,
            tc.tile_pool(name="big", bufs=1) as big,
            tc.tile_pool(name="op", bufs=4) as op,
            tc.tile_pool(name="ps", bufs=4, space="PSUM") as ps,
            tc.tile_pool(name="po", bufs=4, space="PSUM") as po,
        ):
            wkv_sb = wp.tile([P, IT, 2 * CC], io_dt, tag="wkv")
            wq_sb = wp.tile([P, IT, CC], io_dt, tag="wq")
            wo_sb = wp.tile([D, HPC, H], io_dt, tag="wo")
            hc_t = [
                xp.tile([P, M], io_dt, tag="xt", name=f"hc_t{it}")
                for it in range(IT)
            ]
            h_t = [
                xp.tile([P, M], io_dt, tag="xt", name=f"h_t{it}")
                for it in range(IT)
            ]

            # trigger engines: critical loads split sync/scalar; bulk on
            # gpsimd (SWDGE); outputs alternate sync/scalar.
            nc.sync.dma_start(
                out=wkv_sb[:],
                in_=wkvT.rearrange("(it p) c -> p it c", p=P),
            )
            for it in range(IT):
                eng = nc.scalar if it % 2 else nc.sync
                eng.dma_start(
                    out=hc_t[it][:, 0:1024], in_=hcT[it * P:(it + 1) * P, 0:1024]
                )
            for it in range(IT):
                eng = nc.scalar if it % 2 else nc.sync
                eng.dma_start(
                    out=hc_t[it][:, 1024:M], in_=hcT[it * P:(it + 1) * P, 1024:M]
                )
            nc.gpsimd.dma_start(
                out=wq_sb[:], in_=wqT.rearrange("(it p) c -> p it c", p=P)
            )
            for it in range(IT):
                nc.gpsimd.dma_start(out=h_t[it][:], in_=hT[it * P:(it + 1) * P, :])
            nc.gpsimd.dma_start(
                out=wo_sb[:], in_=woT.rearrange("(hh p) j -> p hh j", p=D)
            )

            # persistent intermediates
            kv_sb = big.tile([P, LT, 2 * CC], io_dt, tag="kv")
            q_sb = big.tile([P, DT, M], io_dt, tag="q")
            at_sb = big.tile([D, HPC, D], io_dt, tag="at")
            c_sb = big.tile([P, DT, H], io_dt, tag="c")

            # --- stage P1: fused k|v projection (natural [l, d] layout)
            for lt in range(LT):
                acc = ps.tile([P, 2 * CC], f32, tag="ps")
                for it in range(IT):
                    _mm(
                        nc, acc[:],
                        hc_t[it][:, lt * P:(lt + 1) * P],
                        wkv_sb[:, it, :],
                        start=(it == 0), stop=(it == IT - 1),
                    )
                nc.vector.tensor_copy(kv_sb[:, lt, :], acc[:])

            # --- stage A: AT_h = v_h^T k_h  [dv, du], per head
            for hh in range(HPC):
                acc = ps.tile([D, D], f32, tag="ps")
                for lt in range(LT):
                    _mm(
                        nc, acc[:],
                        kv_sb[:, lt, CC + hh * D:CC + (hh + 1) * D],
                        kv_sb[:, lt, hh * D:(hh + 1) * D],
                        start=(lt == 0), stop=(lt == LT - 1),
                    )
                nc.vector.tensor_copy(at_sb[:, hh, :], acc[:])

            # --- stage C: rows of (A_h Wo_h^T) [du, j]
            for hh in range(HPC):
                prow = (hh % 2) * D
                for jc in range(JC):
                    acc = ps.tile([D, 512], f32, tag="ps")
                    _mm(
                        nc, acc[:],
                        at_sb[:, hh, :],
                        wo_sb[:, hh, jc * 512:(jc + 1) * 512],
                        start=True, stop=True,
                    )
                    nc.vector.tensor_copy(
                        c_sb[prow:prow + D, hh // 2, jc * 512:(jc + 1) * 512],
                        acc[:],
                    )

            # --- stage P2 + O interleaved per 512-column chunk of m
            n_out = 0
            for mc in range(MC):
                for dt_i in range(DT):
                    acc = ps.tile([P, 512], f32, tag="ps")
                    for it in range(IT):
                        _mm(
                            nc, acc[:],
                            wq_sb[:, it, dt_i * P:(dt_i + 1) * P],
                            h_t[it][:, mc * 512:(mc + 1) * 512],
                            start=(it == 0), stop=(it == IT - 1),
                        )
                    nc.vector.tensor_copy(
                        q_sb[:, dt_i, mc * 512:(mc + 1) * 512], acc[:]
                    )
                for mt in range(mc * 4, (mc + 1) * 4):
                    o_t = op.tile([P, H], f32, tag="o")
                    for jc in range(JC):
                        acc = po.tile([P, 512], f32, tag="po")
                        for dt_i in range(DT):
                            _mm(
                                nc, acc[:],
                                q_sb[:, dt_i, mt * P:(mt + 1) * P],
                                c_sb[:, dt_i, jc * 512:(jc + 1) * 512],
                                start=(dt_i == 0), stop=(dt_i == DT - 1),
                            )
                        ceng = nc.vector if jc == 0 else nc.scalar
                        if jc == 0:
                            ceng.tensor_copy(o_t[:, 0:512], acc[:])
                        else:
                            ceng.copy(o_t[:, 512:1024], acc[:])
                    eng = nc.gpsimd if n_out % 2 else nc.sync
                    n_out += 1
                    eng.dma_start(
                        out=outp[mt * P:(mt + 1) * P, :], in_=o_t[:]
                    )

    return nc


_NC_CACHE = {}


def _get_nc():
    if "nc" not in _NC_CACHE:
        _NC_CACHE["nc"] = _build_nc()
    return _NC_CACHE["nc"]


def _cast(a):
    a = np.ascontiguousarray(a)
    if MM_DT == "bf16":
        return a.astype(ml_dtypes.bfloat16)
    if MM_DT == "f16":
        return a.astype(np.float16)
    return a.astype(np.float32)


def make_in_maps(h, h_cache, Wq, Wk, Wv, Wo):
    in_maps = []
    for c in range(N_CORES):
        b, g = divmod(c, 4)
        cols = slice(g * CC, (g + 1) * CC)
        in_maps.append({
            "hT": _cast(h[b].T),
            "hcT": _cast(h_cache[b].T),
            "wqT": _cast(Wq[cols, :].T),
            "wkvT": _cast(np.concatenate(
                [Wk[cols, :].T, Wv[cols, :].T], axis=1)),
            "woT": _cast(Wo[:, cols].T),
        })
    return in_maps


def kernel(h, h_cache, key_pe, Wq, Wk, Wv, Wo, _bass_results=None):
    h = np.asarray(h)
    h_cache = np.asarray(h_cache)
    Wq, Wk, Wv, Wo = (np.asarray(a) for a in (Wq, Wk, Wv, Wo))
    nc = _get_nc()
    in_maps = make_in_maps(h, h_cache, Wq, Wk, Wv, Wo)
    res = run_bass_kernel_spmd(nc, in_maps, list(range(N_CORES)))
    if _bass_results is not None:
        _bass_results.append(res)
    out = np.zeros((B, M, H), np.float32)
    for c in range(N_CORES):
        out[c // 4] += res.results[c]["out"]
    return out


# revision 11
# speedup vs baseline: 1.0720x; 1.0720x over previous
"""Bass/Tile TRN2 kernel for nn_MultiHeadSeqAttention_82789789597729.

Math: the reference's softmax / positional scores are dead code -- its output
is exactly  out = concat_h(q_h @ k_h^T @ v_h) @ Wo^T  with no nonlinearity.
By associativity  q (k^T v)  replaces the [M,M] score matrix with a [D,D]
one, collapsing ~69 GFLOP to ~26 GFLOP.

Sharding: tensor-parallel over heads (4 heads / core) x data-parallel over
batch (B=2) -> 8 cores. Each core computes a full-M partial output for its
head group; the host sums the 4 partials per batch (row-parallel unshard).
"""

import numpy as np
import ml_dtypes

import concourse.bass as bass
import concourse.mybir as mybir
import concourse.tile as tile
from concourse.bass_utils import run_bass_kernel_spmd
from concourse.vector_clock import ScopedClock
import bass_rust

B, M, H, K, D = 2, 2048, 1024, 16, 64
N_CORES = 8
HPC = 4           # heads per core
CC = HPC * D      # 256 local feature columns per core
P = 128

# matmul dtype mode: "f32" (safe), "f32r" (full-rate fp32 storage), "bf16"
MM_DT = "f16"


# --- workaround: this walrus rejects multi-wait Drain instructions, so split
# --- the TileContext exit drain into one single-wait drain per proc.
def _split_drain_and_barrier(self, tick_clock, wait_clock):
    n_procs = len(list(tick_clock.global_clock))
    for p, t in enumerate(tick_clock.global_clock):
        if t <= 0:
            continue
        single = bass_rust.VectorClock(
            [t if i == p else 0 for i in range(n_procs)]
        )
        d = self.nc.sync.drain()
        wait_clock.add_sem_waits(d.ins, ScopedClock({None: single}))
    self.nc.all_engine_barrier()
    popped = self.nc._tile_sem_poison_stack.pop()
    assert popped is self._sem_poison
    self.nc.clear_and_free_semaphores(list(self.sems.allocated().values()))
    self.nc.all_engine_barrier()


# --- workaround: the same walrus caps sync waits at 1 per instruction
# --- (2 for EventSemaphore). Tile's wait-assignment can attach more; hoist
# --- the extras onto single-wait nop carriers emitted just before.
_ORIG_COMMIT_AND_LOWER = tile.TileContext._commit_and_lower


def _wait_split_commit_and_lower(self, inst, original_block, old_bb_map,
                                 bb_to_exit_bb):
    si = inst.sync_info
    cap = 2 if isinstance(inst, mybir.InstEventSemaphore) else 1
    ow = list(si.on_wait) if si is not None and si.on_wait else []
    if len(ow) > cap and inst.is_executable():
        for w in ow[:-cap]:
            carrier = self.nc.engines[inst.engine].nop(nofuse=True)
            carrier.ins.sync_info = bass_rust.SyncInfo(
                on_wait=[w], on_update=[]
            )
        inst.sync_info = bass_rust.SyncInfo(
            on_wait=ow[-cap:], on_update=list(si.on_update or [])
        )
    return _ORIG_COMMIT_AND_LOWER(
        self, inst, original_block, old_bb_map, bb_to_exit_bb
    )


if not getattr(tile.TileContext, "_split_drain_patched", False):
    tile.TileContext._drain_and_barrier = _split_drain_and_barrier
    tile.TileContext._commit_and_lower = _wait_split_commit_and_lower
    tile.TileContext._split_drain_patched = True


def _mm(nc, out, lhsT, rhs, start, stop):
    nc.tensor.matmul(out, lhsT, rhs, start=start, stop=stop)


def _build_nc():
    if MM_DT == "bf16":
        io_dt = mybir.dt.bfloat16
    elif MM_DT == "f16":
        io_dt = mybir.dt.float16
    elif MM_DT == "f32r":
        io_dt = mybir.dt.float32r
    else:
        io_dt = mybir.dt.float32
    f32 = mybir.dt.float32

    nc = bass.Bass()
    hT = nc.dram_tensor("hT", [H, M], io_dt, kind="ExternalInput")
    hcT = nc.dram_tensor("hcT", [H, M], io_dt, kind="ExternalInput")
    wqT = nc.dram_tensor("wqT", [H, CC], io_dt, kind="ExternalInput")
    wkvT = nc.dram_tensor("wkvT", [H, 2 * CC], io_dt, kind="ExternalInput")
    woT = nc.dram_tensor("woT", [CC, H], io_dt, kind="ExternalInput")
    out_dt = mybir.dt.float16 if MM_DT == "f16" else f32
    outp = nc.dram_tensor("out", [M, H], out_dt, kind="ExternalOutput")

    IT = H // P           # 8 contraction tiles over feature dim
    LT = M // P           # 16 tiles over sequence dim
    MC = M // 512         # 4 moving chunks over sequence dim
    DT = CC // P          # 2 partition tiles over local feature cols
    JC = H // 512         # 2 chunks over output feature dim

    x_bufs = 12 if MM_DT in ("bf16", "f16") else 9

    with tile.TileContext(nc) as tc:
        with (
            tc.tile_pool(name="wp", bufs=1) as wp,
            tc.tile_pool(name="xp", bufs=16) as xp,
            tc.tile_pool(name="big", bufs=1) as big,
            tc.tile_pool(name="op", bufs=4) as op,
            tc.tile_pool(name="ps", bufs=4, space="PSUM") as ps,
            tc.tile_pool(name="po", bufs=4, space="PSUM") as po,
        ):
            wkv_sb = wp.tile([P, IT, 2 * CC], io_dt, tag="wkv")
            wq_sb = wp.tile([P, IT, CC], io_dt, tag="wq")
            wo_sb = wp.tile([D, HPC, H], io_dt, tag="wo")
            hc_t = [
                xp.tile([P, M], io_dt, tag="xt", name=f"hc_t{it}")
                for it in range(IT)
            ]
            h_t = [
                xp.tile([P, M], io_dt, tag="xt", name=f"h_t{it}")
                for it in range(IT)
            ]

            # Input DMAs ride sync/scalar/vector queues in priority order
            # (per-engine queues are FIFO, so criticals transfer first);
            # output DMAs go on the otherwise-idle gpsimd queue.
            wkv_r = wkvT.rearrange("(it p) c -> p it c", p=P)
            for half in range(2):
                sl = slice(half * 4, half * 4 + 4)
                eng = nc.scalar if half else nc.sync
                eng.dma_start(out=wkv_sb[:, sl, :], in_=wkv_r[:, sl, :])
            for it in range(IT):
                eng = nc.scalar if it % 2 else nc.sync
                eng.dma_start(
                    out=hc_t[it][:, 0:1024], in_=hcT[it * P:(it + 1) * P, 0:1024]
                )
            for it in range(IT):
                eng = nc.scalar if it % 2 else nc.sync
                eng.dma_start(
                    out=hc_t[it][:, 1024:M], in_=hcT[it * P:(it + 1) * P, 1024:M]
                )
            nc.gpsimd.dma_start(
                out=wq_sb[:], in_=wqT.rearrange("(it p) c -> p it c", p=P)
            )
            nc.gpsimd.dma_start(
                out=wo_sb[:], in_=woT.rearrange("(hh p) j -> p hh j", p=D)
            )
            for it in range(IT):
                eng = nc.scalar if it % 2 else nc.sync
                eng.dma_start(out=h_t[it][:], in_=hT[it * P:(it + 1) * P, :])

            # persistent intermediates
            kv_sb = big.tile([P, LT, 2 * CC], io_dt, tag="kv")
            q_sb = big.tile([P, DT, M], io_dt, tag="q")
            at_sb = big.tile([D, HPC, D], io_dt, tag="at")
            c_sb = big.tile([P, DT, H], io_dt, tag="c")

            # --- stage P1: fused k|v projection (natural [l, d] layout)
            for lt in range(LT):
                acc = ps.tile([P, 2 * CC], f32, tag="ps")
                for it in range(IT):
                    _mm(
                        nc, acc[:],
                        hc_t[it][:, lt * P:(lt + 1) * P],
                        wkv_sb[:, it, :],
                        start=(it == 0), stop=(it == IT - 1),
                    )
                nc.vector.tensor_copy(kv_sb[:, lt, :], acc[:])

            # --- stage A: AT_h = v_h^T k_h  [dv, du], per head
            for hh in range(HPC):
                acc = ps.tile([D, D], f32, tag="ps")
                for lt in range(LT):
                    _mm(
                        nc, acc[:],
                        kv_sb[:, lt, CC + hh * D:CC + (hh + 1) * D],
                        kv_sb[:, lt, hh * D:(hh + 1) * D],
                        start=(lt == 0), stop=(lt == LT - 1),
                    )
                nc.vector.tensor_copy(at_sb[:, hh, :], acc[:])

            # --- stage C: rows of (A_h Wo_h^T) [du, j]
            for hh in range(HPC):
                prow = (hh % 2) * D
                for jc in range(JC):
                    acc = ps.tile([D, 512], f32, tag="ps")
                    _mm(
                        nc, acc[:],
                        at_sb[:, hh, :],
                        wo_sb[:, hh, jc * 512:(jc + 1) * 512],
                        start=True, stop=True,
                    )
                    nc.vector.tensor_copy(
                        c_sb[prow:prow + D, hh // 2, jc * 512:(jc + 1) * 512],
                        acc[:],
                    )

            # --- stage P2 + O interleaved per 512-column chunk of m
            n_out = 0
            for mc in range(MC):
                for dt_i in range(DT):
                    acc = ps.tile([P, 512], f32, tag="ps")
                    for it in range(IT):
                        _mm(
                            nc, acc[:],
                            wq_sb[:, it, dt_i * P:(dt_i + 1) * P],
                            h_t[it][:, mc * 512:(mc + 1) * 512],
                            start=(it == 0), stop=(it == IT - 1),
                        )
                    nc.vector.tensor_copy(
                        q_sb[:, dt_i, mc * 512:(mc + 1) * 512], acc[:]
                    )
                for mt in range(mc * 4, (mc + 1) * 4):
                    o_t = op.tile([P, H], out_dt, tag="o")
                    for jc in range(JC):
                        acc = po.tile([P, 512], f32, tag="po")
                        for dt_i in range(DT):
                            _mm(
                                nc, acc[:],
                                q_sb[:, dt_i, mt * P:(mt + 1) * P],
                                c_sb[:, dt_i, jc * 512:(jc + 1) * 512],
                                start=(dt_i == 0), stop=(dt_i == DT - 1),
                            )
                        nc.vector.tensor_copy(
                            o_t[:, jc * 512:(jc + 1) * 512], acc[:]
                        )
                    n_out += 1
                    nc.gpsimd.dma_start(
                        out=outp[mt * P:(mt + 1) * P, :], in_=o_t[:]
                    )

    return nc


_NC_CACHE = {}


def _get_nc():
    if "nc" not in _NC_CACHE:
        _NC_CACHE["nc"] = _build_nc()
    return _NC_CACHE["nc"]


def _cast(a):
    a = np.ascontiguousarray(a)
    if MM_DT == "bf16":
        return a.astype(ml_dtypes.bfloat16)
    if MM_DT == "f16":
        return a.astype(np.float16)
    return a.astype(np.float32)


def make_in_maps(h, h_cache, Wq, Wk, Wv, Wo):
    in_maps = []
    for c in range(N_CORES):
        b, g = divmod(c, 4)
        cols = slice(g * CC, (g + 1) * CC)
        in_maps.append({
            "hT": _cast(h[b].T),
            "hcT": _cast(h_cache[b].T),
            "wqT": _cast(Wq[cols, :].T),
            "wkvT": _cast(np.concatenate(
                [Wk[cols, :].T, Wv[cols, :].T], axis=1)),
            "woT": _cast(Wo[:, cols].T),
        })
    return in_maps


def kernel(h, h_cache, key_pe, Wq, Wk, Wv, Wo, _bass_results=None):
    h = np.asarray(h)
    h_cache = np.asarray(h_cache)
    Wq, Wk, Wv, Wo = (np.asarray(a) for a in (Wq, Wk, Wv, Wo))
    nc = _get_nc()
    in_maps = make_in_maps(h, h_cache, Wq, Wk, Wv, Wo)
    res = run_bass_kernel_spmd(nc, in_maps, list(range(N_CORES)))
    if _bass_results is not None:
        _bass_results.append(res)
    out = np.zeros((B, M, H), np.float32)
    for c in range(N_CORES):
        out[c // 4] += res.results[c]["out"].astype(np.float32)
    return out
